# revision 20
# baseline (speedup 1.0000x reference)
"""AttentionOnAttention Trainium2 kernel v2 (8 NeuronCores, SPMD).

Sharding: core c handles batch b = c//4 and heads [4*(c%4), 4*(c%4)+4);
each core computes the disjoint output slice out[b, :, 256*(c%4):...] so no
collectives are needed.

v2 design (ACT exp stream is the wall at ~147us; everything else hides
under it):
  - S matmuls in fp8e4m3 DoubleRow (0.5 cyc/row): q/k quantized per-head
    into [32, 2, N] K-packed layout (d = t*32 + dlo), S = one DR matmul
    per (head, chunk, jtile) at tile_position (h*32, 0).
  - k/v projections in fp8-DR too (host-packed x-hat [64p, kt, 2, N] with
    d = kt*128 + t*64 + p, and matching W-hat), q projection stays bf16
    (feeds the AoA linears directly; fp8 noise there would exceed the
    rel-err gate). PE stream drops ~157us -> ~120us, below the ACT wall.
  - exp tiles stay [128, 1024] (2 heads x 512 i) = (1024+352)/1.2GHz each.
  - ALL projections + AoA + finals are emitted as cost-budgeted filler
    micros inside the exp-stream j-loops; PV emission "chases" v-tile
    production through a per-chunk chaser (PV accumulation order over j is
    commutative) so the first chunk tolerates v not being ready.
  - finals use sigmoid(x) = 0.5*(1+tanh(x/2)) with the 0.5 folded into the
    I-half weights/bias host-side: tanh lives in exp's ACT table set, so
    the whole kernel needs ONE table load and finals can sit mid-stream.
  - DVE reciprocal -> reciprocal_approx_fast (51 ULP, ~5x faster).
"""

import numpy as np
from collections import deque
from contextlib import ExitStack

import concourse.bass as bass
import concourse.bacc as bacc
import concourse.tile as tile
from concourse import mybir

B, N, DIM, H, DH = 2, 2048, 1024, 16, 64
HPC = H // 4          # 4 heads per core
INC = HPC * DH        # 256 per-core inner width
KT = DIM // 128       # 8 contraction tiles (bf16) / DR-packed fp8 tiles
NCH = N // 512        # 4 free-dim chunks of 512
JT = N // 128         # 16 j tiles
SCALE = float(DH) ** -0.5
F32 = mybir.dt.float32
BF16 = mybir.dt.bfloat16
E4 = mybir.dt.float8e4
AF = mybir.ActivationFunctionType
ALU = mybir.AluOpType
PM = mybir.MatmulPerfMode

STEP_BUDGET = 1000.0  # ns of PE work per exp step (exp = 1147ns)


def build_nc():
    nc = bacc.Bacc(
        "TRN2",
        target_bir_lowering=False,
        debug=False,
        enable_asserts=False,
        num_devices=8,
    )
    xt_d = nc.dram_tensor("xt", (KT, 128, N), BF16, kind="ExternalInput").ap()
    xp8_d = nc.dram_tensor("xp8", (64, KT, 2, N), E4, kind="ExternalInput").ap()
    wq_d = nc.dram_tensor("wq", (128, KT, 2, 128), BF16, kind="ExternalInput").ap()
    wk8_d = nc.dram_tensor("wk8", (64, KT, 2, 2, 128), E4, kind="ExternalInput").ap()
    wv8_d = nc.dram_tensor("wv8", (64, KT, 2, 256), E4, kind="ExternalInput").ap()
    wcq4_d = nc.dram_tensor("wcq4", (128, 2, 128), BF16, kind="ExternalInput").ap()
    wca_d = nc.dram_tensor("wca", (DH, 2 * DH), BF16, kind="ExternalInput").ap()
    bias_d = nc.dram_tensor("biases", (2 * DH, 1), F32, kind="ExternalInput").ap()
    outT_d = nc.dram_tensor("outT", (INC, N), BF16, kind="ExternalOutput").ap()
    dbg_aoU_d = nc.dram_tensor("dbg_aoU", (DH + 1, N), F32, kind="ExternalOutput").ap()
    dbg_aoT_d = nc.dram_tensor("dbg_aoT", (DH, N), BF16, kind="ExternalOutput").ap()
    dbg_q_d = nc.dram_tensor("dbg_q", (128, 2, N), BF16, kind="ExternalOutput").ap()

    with tile.TileContext(nc) as tc, ExitStack() as ctx:
        consts = ctx.enter_context(tc.tile_pool(name="consts", bufs=1))
        psum = ctx.enter_context(tc.tile_pool(name="psum", bufs=2, space="PSUM"))
        esp = ctx.enter_context(tc.tile_pool(name="es_p", bufs=12))
        nrm = ctx.enter_context(tc.tile_pool(name="norm_p", bufs=4))
        fin = ctx.enter_context(tc.tile_pool(name="fin_p", bufs=2))

        # ---------------- persistent tensors ----------------
        q_bf = consts.tile([128, 2, N], BF16, name="q_bf")
        q8 = consts.tile([128, 2, N], E4, name="q8")
        k8 = consts.tile([128, 2, N], E4, name="k8")
        v_aug = consts.tile([128, JT, HPC, DH + 1], BF16, name="v_aug")
        nc.vector.memset(v_aug[:, :, :, DH : DH + 1], 1.0)
        aoT = [consts.tile([DH, N], BF16, name=f"aoT{h}") for h in range(HPC)]
        aoU = [consts.tile([DH + 1, N], F32, name=f"aoU{h}") for h in range(HPC)]

        # tiny AoA weights + bias first so the ACT warm can run during the
        # big DMAs
        wcq4_sb = consts.tile([128, 2, 128], BF16, name="wcq4_sb")
        nc.sync.dma_start(out=wcq4_sb, in_=wcq4_d)
        wca_sb = consts.tile([DH, 2 * DH], BF16, name="wca_sb")
        nc.sync.dma_start(out=wca_sb, in_=wca_d)
        bias_sb = consts.tile([2 * DH, 1], F32, name="bias_sb")
        nc.sync.dma_start(out=bias_sb, in_=bias_d)

        xw = tc.alloc_tile_pool(name="xw", bufs=1)
        wq_sb = xw.tile([128, KT, 2, 128], BF16, name="wq_sb")
        wk8_sb = xw.tile([64, KT, 2, 2, 128], E4, name="wk8_sb")
        wv8_sb = xw.tile([64, KT, 2, 256], E4, name="wv8_sb")
        xt_sb = xw.tile([128, KT, N], BF16, name="xt_sb")
        xp8_sb = xw.tile([64, KT, 2, N], E4, name="xp8_sb")
        nc.sync.dma_start(out=wq_sb, in_=wq_d)
        nc.sync.dma_start(out=wk8_sb, in_=wk8_d)
        # chunk-0 bf16 x columns + ALL fp8 x columns first: chunk 0's S
        # scans every j-tile, so the k projection of all 4 chunks (and the
        # full v) must be producible during chunk 0's steps.
        for k in range(KT):
            nc.sync.dma_start(out=xt_sb[:, k, 0:512], in_=xt_d[k, :, 0:512])
            nc.sync.dma_start(out=xp8_sb[:, k, :, 0:512], in_=xp8_d[:, k, :, 0:512])
        nc.sync.dma_start(out=wv8_sb, in_=wv8_d)
        for k in range(KT):
            nc.sync.dma_start(out=xp8_sb[:, k, :, 512:N], in_=xp8_d[:, k, :, 512:N])
        for k in range(KT):
            nc.sync.dma_start(out=xt_sb[:, k, 512:N], in_=xt_d[k, :, 512:N])

        # one ACT table load (exp_and_others: has Exp AND Tanh)
        warm_sb = consts.tile([128, 1], F32, name="warm_sb")
        nc.scalar.activation(out=warm_sb, in_=bias_sb, func=AF.Exp)
        nc.scalar.activation(out=warm_sb, in_=warm_sb, func=AF.Tanh)

        # ---------------- filler micro units ----------------
        k_done, q_done, aoT_done = set(), set(), set()
        def q_micros(c):
            """bf16 q projection for chunk c -> q_bf + q8 (both layouts:
            partition = h*32+dlo, free dim0 = dhi)."""
            cs = slice(c * 512, (c + 1) * 512)
            st = {}

            def mk(kt, t):
                def _mm():
                    if "ps" not in st:
                        st["ps"] = psum.tile(
                            [128, 1024], F32, name="qps", tag="pj", bufs=1
                        )
                    nc.tensor.matmul(
                        st["ps"][:, t * 512 : (t + 1) * 512],
                        lhsT=wq_sb[:, kt, t, :],
                        rhs=xt_sb[:, kt, cs],
                        start=(kt == 0),
                        stop=(kt == KT - 1),
                    )
                return _mm

            def _evac_bf():
                nc.vector.tensor_copy(
                    out=q_bf[:, :, cs],
                    in_=st["ps"].rearrange("p (a b) -> p a b", a=2),
                )

            def _evac_8():
                nc.vector.tensor_copy(
                    out=q8[:, :, cs],
                    in_=st["ps"].rearrange("p (a b) -> p a b", a=2),
                )
                q_done.add(c)

            return [(213, mk(kt, t)) for t in range(2) for kt in range(KT)] + [
                (30, _evac_bf),
                (30, _evac_8),
            ]

        def k_micros(c):
            """fp8-DR k projection for chunk c -> k8."""
            cs = slice(c * 512, (c + 1) * 512)
            st = {}

            def mk(kt, dhi):
                def _mm():
                    if "ps" not in st:
                        st["ps"] = psum.tile(
                            [128, 1024], F32, name="kps", tag="pj", bufs=1
                        )
                    nc.tensor.matmul(
                        st["ps"][:, dhi * 512 : (dhi + 1) * 512],
                        lhsT=wk8_sb[:, kt, :, dhi, :],
                        rhs=xp8_sb[:, kt, :, cs],
                        start=(kt == 0),
                        stop=(kt == KT - 1),
                        perf_mode=PM.DoubleRow,
                        tile_position=(0, 0),
                    )
                return _mm

            def _evac():
                nc.vector.tensor_copy(
                    out=k8[:, :, cs],
                    in_=st["ps"].rearrange("p (a b) -> p a b", a=2),
                )
                k_done.add(c)

            return [(107, mk(kt, dhi)) for dhi in range(2) for kt in range(KT)] + [
                (30, _evac)
            ]

        def v_micros(it):
            """fp8-DR v projection for token j-tile `it` -> v_aug."""
            st = {}

            def mk(kt):
                def _mm():
                    if "ps" not in st:
                        st["ps"] = psum.tile(
                            [128, 256], F32, name="vps", tag="pj", bufs=1
                        )
                    nc.tensor.matmul(
                        st["ps"],
                        lhsT=xp8_sb[:, kt, :, it * 128 : (it + 1) * 128],
                        rhs=wv8_sb[:, kt, :, :],
                        start=(kt == 0),
                        stop=(kt == KT - 1),
                        perf_mode=PM.DoubleRow,
                        tile_position=(0, 0),
                    )
                return _mm

            def _evac():
                nc.vector.tensor_copy(
                    out=v_aug[:, it, :, 0:DH],
                    in_=st["ps"].rearrange("p (h d) -> p h d", h=HPC),
                )
                v_ready[it] = True

            return [(53, mk(kt)) for kt in range(KT)] + [(30, _evac)]

        def aoa_mms(st, h, half, cc, tag, tag_bufs):
            """The 3 AoA matmuls for (head h, half, sub-chunk cc)."""
            c = 2 * half + cc
            cs = slice(c * 512, (c + 1) * 512)

            def mk(which):
                def _mm():
                    if "ig" not in st:
                        st["ig"] = psum.tile(
                            [128, 1024], F32, name="ig", tag=tag, bufs=tag_bufs
                        )
                    igs = st["ig"][:, cc * 512 : (cc + 1) * 512]
                    assert c in q_done, f"aoa h{h} reads q chunk {c} unwritten"
                    if which == 2:
                        assert (h, c) in aoT_done, f"aoa reads aoT[{h}] c{c} unwritten"
                    if which < 2:
                        nc.tensor.matmul(
                            igs,
                            lhsT=wcq4_sb[h * 32 : (h + 1) * 32, which, :],
                            rhs=q_bf[h * 32 : (h + 1) * 32, which, cs],
                            start=(which == 0),
                            stop=False,
                            tile_position=(h * 32, 0),
                        )
                    else:
                        nc.tensor.matmul(
                            igs,
                            lhsT=wca_sb,
                            rhs=aoT[h][:, cs],
                            start=False,
                            stop=True,
                            tile_position=(0, 0),
                        )
                return _mm

            return [(213, mk(w)) for w in range(3)]

        def final_micros(st, h, half):
            """tanh-gate + STT + output DMA for (head h, half)."""
            hs = slice(half * 1024, (half + 1) * 1024)

            def _tanh():
                st["y"] = fin.tile([DH, 1024], BF16, name="y", tag="y")
                nc.scalar.activation(
                    out=st["y"],
                    in_=st["ig"][DH:128, :],
                    func=AF.Tanh,
                    scale=0.5,
                    bias=bias_sb[DH : 2 * DH, :],
                )

            def _add1():
                st["y1"] = fin.tile([DH, 1024], BF16, name="y1", tag="y1")
                nc.vector.tensor_scalar_add(out=st["y1"], in0=st["y"], scalar1=1.0)

            def _stt():
                ot = fin.tile([DH, 1024], BF16, name="ot", tag="ot")
                nc.vector.scalar_tensor_tensor(
                    out=ot,
                    in0=st["ig"][0:DH, :],
                    scalar=bias_sb[0:DH, :],
                    in1=st["y1"],
                    op0=ALU.add,
                    op1=ALU.mult,
                )
                nc.sync.dma_start(out=outT_d[h * DH : (h + 1) * DH, hs], in_=ot)

            return [(50, _tanh), (30, _add1), (30, _stt)]

        def aoa_final_micros(h, half):
            st = {}
            return (
                aoa_mms(st, h, half, 0, "pj", 1)
                + aoa_mms(st, h, half, 1, "pj", 1)
                + final_micros(st, h, half)
            )

        def norm_chain(h, c):
            """aoT[h][:, chunk c] = ao_unnorm / L, off the critical path."""
            cs = slice(c * 512, (c + 1) * 512)
            rl = nrm.tile([1, 512], F32, name="rl", tag="rl")
            nc.vector.reciprocal(out=rl, in_=aoU[h][DH : DH + 1, cs])
            rlb = nrm.tile([DH, 512], F32, name="rlb", tag="rlb")
            nc.gpsimd.partition_broadcast(rlb, rl)
            nc.vector.tensor_mul(out=aoT[h][:, cs], in0=aoU[h][0:DH, cs], in1=rlb)
            aoT_done.add((h, c))

        # ---------------- pv chaser ----------------
        v_ready = [False] * JT

        class Chaser:
            """Emits the PV accumulation for one (pair, chunk) lazily: PV(jt)
            can only be emitted once exp(jt) was emitted AND v tile jt was
            produced. PSUM accumulation over j is order-independent, so a
            chunk's PV train may spill into later chunks' steps."""

            def __init__(self, p, c):
                self.p, self.c = p, c
                self.next = 0
                self.n_exp = 0
                self.es_tiles = [None] * JT
                self.pv = None

            def emit_one(self):
                jt = self.next
                if self.pv is None:
                    # lazy: the pv ring slots recycle the previous chunk's
                    # tiles, whose evacs are only emitted at that chaser's
                    # finish() — allocate after it, not at chunk start
                    self.pv = [
                        psum.tile(
                            [DH + 1, 512], F32, name=f"pv{hh}", tag="pv", bufs=2
                        )
                        for hh in range(2)
                    ]
                for hh in range(2):
                    nc.tensor.matmul(
                        self.pv[hh],
                        lhsT=v_aug[:, jt, 2 * self.p + hh, :],
                        rhs=self.es_tiles[jt][:, hh * 512 : (hh + 1) * 512],
                        start=(jt == 0),
                        stop=(jt == JT - 1),
                    )
                self.es_tiles[jt] = None
                self.next += 1
                return 427

            def runnable(self):
                return (
                    self.next < self.n_exp
                    and self.next < JT
                    and v_ready[self.next]
                )

            def done(self):
                return self.next >= JT

            def finish(self):
                """Evacuate PSUM (both heads), the slow norm chains, then
                queue any AoA+finals units whose aoT inputs just became
                fully emitted (emission order is logical order)."""
                cs = slice(self.c * 512, (self.c + 1) * 512)
                for hh in range(2):
                    nc.vector.tensor_copy(
                        out=aoU[2 * self.p + hh][:, cs], in_=self.pv[hh]
                    )
                for hh in range(2):
                    norm_chain(2 * self.p + hh, self.c)
                if self.c % 2 == 1:  # half = c//2 complete for this pair
                    half = self.c // 2
                    for hh in range(2):
                        h = 2 * self.p + hh
                        add_unit(f"af{h}{half}", aoa_final_micros(h, half))

        chasers = deque()
        fillers = deque()
        emitted = set()

        def add_unit(name, micros):
            fillers.extend(micros)
            fillers.append((0, lambda: emitted.add(name)))

        def require(*names):
            """Emission order IS logical order for SBUF writes: force-pop
            fillers until the named units' writes are emitted."""
            for name in names:
                while name not in emitted:
                    if not fillers:
                        raise AssertionError(f"require({name}): fillers empty")
                    fillers.popleft()[1]()

        def es_outstanding():
            return sum(ch.n_exp - ch.next for ch in chasers)

        def force_chase_one():
            """Force the oldest chaser forward one PV (emitting whatever
            filler units gate it) so es-ring slots are provably consumed
            before being reallocated."""
            ch = chasers[0]
            if ch.done():
                ch.finish()
                chasers.popleft()
                return
            while not v_ready[ch.next]:
                if not fillers:
                    raise AssertionError("es forcing: fillers empty")
                fillers.popleft()[1]()
            ch.emit_one()

        def service(budget):
            while budget > 0 and chasers:
                ch = chasers[0]
                if ch.done():
                    ch.finish()
                    chasers.popleft()
                    continue
                if not ch.runnable():
                    break
                budget -= ch.emit_one()
            while fillers and budget >= fillers[0][0]:
                cost, fn = fillers.popleft()
                fn()
                budget -= cost
            return budget

        # ---------------- prologue: chunk-0 q + k inline ----------------
        for _, fn in q_micros(0) + k_micros(0):
            fn()

        # ---------------- main exp stream ----------------
        for p in range(2):
            if p == 1:
                # all projection fillers reference xw tensors; force-drain
                # any stragglers before releasing the pool
                while fillers:
                    fillers.popleft()[1]()
                xw.release()
            for c in range(NCH):
                if p == 0 and c == 0:
                    # priority: k chunks gate chunk-0's own exps (S scans all
                    # j); q(1) gates chunk 1's first exp; v only gates the
                    # PV chase, which may lag (es backlog absorbs it)
                    add_unit("k1", k_micros(1))
                    for it in range(4):
                        add_unit(f"v{it}", v_micros(it))
                    add_unit("k2", k_micros(2))
                    for it in range(4, 8):
                        add_unit(f"v{it}", v_micros(it))
                    add_unit("k3", k_micros(3))
                    add_unit("q1", q_micros(1))
                    for it in range(8, JT):
                        add_unit(f"v{it}", v_micros(it))
                elif p == 0 and c < 3:
                    add_unit(f"q{c + 1}", q_micros(c + 1))

                if p == 0 and c > 0:
                    require(f"q{c}")
                ch = Chaser(p, c)
                chasers.append(ch)
                cs = slice(c * 512, (c + 1) * 512)
                for jt in range(JT):
                    if p == 0 and jt % 4 == 0 and jt > 0:
                        require(f"k{jt // 4}")
                    assert c in q_done, f"S p{p} c{c} reads q chunk {c} unwritten"
                    assert jt // 4 in k_done, f"S p{p} c{c} jt{jt} reads k unwritten"
                    jts = slice(jt * 128, (jt + 1) * 128)
                    s = psum.tile([128, 1024], F32, name="s", tag="s", bufs=2)
                    for hh in range(2):
                        h = 2 * p + hh
                        nc.tensor.matmul(
                            s[:, hh * 512 : (hh + 1) * 512],
                            lhsT=k8[h * 32 : (h + 1) * 32, :, jts],
                            rhs=q8[h * 32 : (h + 1) * 32, :, cs],
                            start=True,
                            stop=True,
                            perf_mode=PM.DoubleRow,
                            tile_position=(h * 32, 0),
                        )
                    while chasers and es_outstanding() >= 11:
                        force_chase_one()
                    es = esp.tile([128, 1024], BF16, name="es", tag="es")
                    nc.scalar.activation(out=es, in_=s, func=AF.Exp, scale=SCALE)
                    ch.es_tiles[jt] = es
                    ch.n_exp = jt + 1
                    service(STEP_BUDGET - 213)

        # ---------------- drain + tail ----------------
        # Chaser-priority drain (same order as service()): fillers may
        # contain v evacs that unblock chasers, and aoa fillers must not
        # jump ahead of the norm chains emitted by chaser.finish().
        while chasers or fillers:
            progress = False
            while chasers:
                ch = chasers[0]
                if ch.done():
                    ch.finish()
                    chasers.popleft()
                    progress = True
                    continue
                if ch.runnable():
                    ch.emit_one()
                    progress = True
                    continue
                break
            if fillers:
                fillers.popleft()[1]()
                progress = True
            if not progress:
                raise AssertionError("drain deadlock")

        nc.sync.dma_start(out=dbg_aoU_d, in_=aoU[0])
        nc.sync.dma_start(out=dbg_aoT_d, in_=aoT[0])
        nc.sync.dma_start(out=dbg_q_d, in_=q_bf)
    nc.compile()
    return nc


_NC_CACHE = None


def _get_nc():
    global _NC_CACHE
    if _NC_CACHE is None:
        _NC_CACHE = build_nc()
    return _NC_CACHE


def make_in_maps(x, Wq, Wkv, Wq_out, Wattn_out, out_bias, Wq_gate, Wattn_gate,
                 gate_bias):
    import ml_dtypes

    bf16 = ml_dtypes.bfloat16
    e4 = ml_dtypes.float8_e4m3fn
    # sigmoid(x) = 0.5*(1+tanh(x/2)): fold the 0.5 into the I-half weights
    # and bias; G-half raw (ACT applies scale=0.5 and the pre-halved bias).
    wcq_s = np.concatenate([0.5 * Wq_out.T, Wq_gate.T], axis=1)  # [64, 128]
    wca = np.ascontiguousarray(
        np.concatenate([0.5 * Wattn_out.T, Wattn_gate.T], axis=1), dtype=bf16
    )
    wcq4 = np.ascontiguousarray(
        np.broadcast_to(
            wcq_s.reshape(2, 32, 128).transpose(1, 0, 2)[None], (4, 32, 2, 128)
        ).reshape(128, 2, 128),
        dtype=bf16,
    )
    biases = 0.5 * np.concatenate(
        [out_bias.reshape(-1), gate_bias.reshape(-1)]
    ).astype(np.float32).reshape(2 * DH, 1)
    biases = np.ascontiguousarray(biases)

    Wk = Wkv[:, : H * DH]
    Wv = Wkv[:, H * DH :]
    xT = [np.ascontiguousarray(x[b].T) for b in range(B)]
    xt = [a.reshape(KT, 128, N).astype(bf16) for a in xT]
    # x-hat: [p, kt, t, n] = xT[kt*128 + t*64 + p, n]
    xp8 = [
        np.ascontiguousarray(a.reshape(KT, 2, 64, N).transpose(2, 0, 1, 3)).astype(e4)
        for a in xT
    ]
    in_maps = []
    for core in range(8):
        b, hg = core // 4, core % 4
        cols = slice(hg * INC, (hg + 1) * INC)
        Wq_c, Wk_c, Wv_c = Wq[:, cols], Wk[:, cols], Wv[:, cols]
        # wq: [p, kt, t, h*32+dlo] = Wq_c[kt*128+p, h*64+t*32+dlo]
        wq = np.ascontiguousarray(
            Wq_c.reshape(KT, 128, 4, 2, 32)
            .transpose(1, 0, 3, 2, 4)
            .reshape(128, KT, 2, 128)
        ).astype(bf16)
        # wk8: [p, kt, t, dhi, h*32+dlo] = Wk_c[kt*128+t*64+p, h*64+dhi*32+dlo]
        wk8 = np.ascontiguousarray(
            Wk_c.reshape(KT, 2, 64, 4, 2, 32)
            .transpose(2, 0, 1, 4, 3, 5)
            .reshape(64, KT, 2, 2, 128)
        ).astype(e4)
        # wv8: [p, kt, t, n] = Wv_c[kt*128+t*64+p, n]
        wv8 = np.ascontiguousarray(
            Wv_c.reshape(KT, 2, 64, 256).transpose(2, 0, 1, 3)
        ).astype(e4)
        in_maps.append(
            {
                "xt": xt[b],
                "xp8": xp8[b],
                "wq": wq,
                "wk8": wk8,
                "wv8": wv8,
                "wcq4": wcq4,
                "wca": wca,
                "biases": biases,
            }
        )
    return in_maps


def assemble_output(results):
    out = np.empty((B, N, H * DH), dtype=np.float32)
    for c in range(8):
        b, hg = c // 4, c % 4
        out[b, :, hg * INC : (hg + 1) * INC] = results[c]["outT"].T.astype(np.float32)
    return out


def kernel(**inputs):
    from concourse.bass_utils import run_bass_kernel_spmd

    inputs = {k: np.asarray(v, dtype=np.float32) for k, v in inputs.items()}
    nc = _get_nc()
    in_maps = make_in_maps(**inputs)
    res = run_bass_kernel_spmd(nc, in_maps, core_ids=list(range(8)))
    return assemble_output(res.results)


# revision 22
# speedup vs baseline: 1.0501x; 1.0501x over previous
"""AttentionOnAttention Trainium2 kernel v3 (8 NeuronCores, SPMD).

Sharding: core c handles batch b = c//4 and heads [4*(c%4), 4*(c%4)+4);
each core computes the disjoint output slice out[b, :, 256*(c%4):...] so no
collectives are needed.

Design: the ACT exp stream (128 x [128,1024] ACTIVATEs ~= 147us) and the
bf16 PE stream (~157us + overheads) are nearly balanced; total time ~=
max of the two plus edges. So: start the exp stream as early as possible
and emit EVERY other piece of work (projections, AoA linears, finals,
PV accumulation) inside the stream's j-loops.

  - bf16 matmuls only. (fp8 DoubleRow was tried: it halves matmul cycles
    on paper but trips HAM throttle class 1 (50% util cap) for ~80% of the
    run, making everything slower. Do not resurrect it.)
  - S pairs: two K=64 matmuls row-packed via tile_position (0,0)/(64,0),
    exp processes the [128, 1024] pair tile in one ACTIVATE.
  - PV emission "chases" v-tile production through per-chunk chasers (PSUM
    accumulation over j is commutative), so chunk 0 tolerates v tiles
    being produced as fillers inside its own steps.
  - Emission order IS logical order for SBUF reuse: require()/flag
    asserts force projection units to be emitted before their consumers,
    and an es-ring guard forces PV progress before es slots recycle.
  - finals use sigmoid(x) = 0.5*(1+tanh(x/2)) with the 0.5 folded into the
    I-half weights/bias host-side: tanh lives in exp's ACT table set, so
    ONE table load total and finals can sit mid-stream.
"""

import numpy as np
from collections import deque
from contextlib import ExitStack

import concourse.bass as bass
import concourse.bacc as bacc
import concourse.tile as tile
from concourse import mybir

B, N, DIM, H, DH = 2, 2048, 1024, 16, 64
HPC = H // 4          # 4 heads per core
INC = HPC * DH        # 256 per-core inner width
KT = DIM // 128       # 8 contraction tiles
NCH = N // 512        # 4 free-dim chunks of 512
JT = N // 128         # 16 j tiles
SCALE = float(DH) ** -0.5
F32 = mybir.dt.float32
BF16 = mybir.dt.bfloat16
AF = mybir.ActivationFunctionType
ALU = mybir.AluOpType

STEP_BUDGET = 1050.0  # ns of PE work per exp step (exp ~= 1147ns)


def build_nc():
    nc = bacc.Bacc(
        "TRN2",
        target_bir_lowering=False,
        debug=False,
        enable_asserts=False,
        num_devices=8,
    )
    xt_d = nc.dram_tensor("xT", (KT, 128, N), BF16, kind="ExternalInput").ap()
    wqkv_d = nc.dram_tensor("wqkv", (DIM, 3 * INC), BF16, kind="ExternalInput").ap()
    wcq_d = nc.dram_tensor("wcq", (DH, 2 * DH), BF16, kind="ExternalInput").ap()
    wca_d = nc.dram_tensor("wca", (DH, 2 * DH), BF16, kind="ExternalInput").ap()
    bias_d = nc.dram_tensor("biases", (2 * DH, 1), F32, kind="ExternalInput").ap()
    outT_d = nc.dram_tensor("outT", (INC, N), BF16, kind="ExternalOutput").ap()

    with tile.TileContext(nc) as tc, ExitStack() as ctx:
        consts = ctx.enter_context(tc.tile_pool(name="consts", bufs=1))
        psum = ctx.enter_context(tc.tile_pool(name="psum", bufs=2, space="PSUM"))
        esp = ctx.enter_context(tc.tile_pool(name="es_p", bufs=12))
        nrm = ctx.enter_context(tc.tile_pool(name="norm_p", bufs=4))
        fin = ctx.enter_context(tc.tile_pool(name="fin_p", bufs=2))

        # ---------------- persistent tensors ----------------
        qTp = [consts.tile([128, N], BF16, name=f"qTp{p}") for p in range(2)]
        qTo = [consts.tile([DH, N], BF16, name=f"qTo{p}") for p in range(2)]
        kTp = [consts.tile([128, N], BF16, name=f"kTp{p}") for p in range(2)]
        v_aug = consts.tile([128, JT, HPC, DH + 1], BF16, name="v_aug")
        nc.vector.memset(v_aug[:, :, :, DH : DH + 1], 1.0)
        aoT = [consts.tile([DH, N], BF16, name=f"aoT{h}") for h in range(HPC)]
        aoU = [consts.tile([DH + 1, N], F32, name=f"aoU{h}") for h in range(HPC)]

        # tiny AoA weights + bias first so the ACT warm runs during big DMAs
        wcq_sb = consts.tile([DH, 2 * DH], BF16, name="wcq_sb")
        nc.sync.dma_start(out=wcq_sb, in_=wcq_d)
        wca_sb = consts.tile([DH, 2 * DH], BF16, name="wca_sb")
        nc.sync.dma_start(out=wca_sb, in_=wca_d)
        bias_sb = consts.tile([2 * DH, 1], F32, name="bias_sb")
        nc.sync.dma_start(out=bias_sb, in_=bias_d)

        wqkv_sb = consts.tile([128, KT, 3 * INC], BF16, name="wqkv_sb")
        wq_sb = wqkv_sb[:, :, 0:INC]
        wk_sb = wqkv_sb[:, :, INC : 2 * INC]
        wv_sb = wqkv_sb[:, :, 2 * INC : 3 * INC]
        xt_sb = consts.tile([128, KT, N], BF16, name="xt_sb")
        # weights first, then chunk-0 x columns, then the rest (pair-0
        # chunk-0's S scans all j so all x columns are needed early-ish)
        for k in range(KT):
            ks = slice(k * 128, (k + 1) * 128)
            nc.sync.dma_start(out=wqkv_sb[:, k, :], in_=wqkv_d[ks, :])
        for cc in range(NCH):
            ccs = slice(cc * 512, (cc + 1) * 512)
            for k in range(KT):
                nc.sync.dma_start(out=xt_sb[:, k, ccs], in_=xt_d[k, :, ccs])

        # one ACT table load (exp_and_others has both Exp and Tanh)
        warm_sb = consts.tile([128, 1], F32, name="warm_sb")
        nc.scalar.activation(out=warm_sb, in_=bias_sb, func=AF.Exp)
        nc.scalar.activation(out=warm_sb, in_=warm_sb, func=AF.Tanh)

        # ---------------- filler micro units ----------------
        qk_done, v_ready, aoT_done = set(), [False] * JT, set()

        def qk_micros(kind, p, c):
            """Pair projection (baseline form): 8 K=128 matmuls -> [128 =
            2 heads x 64d, 512] PSUM, then copy to qTp/kTp (+qTo)."""
            wsb = wq_sb if kind == "q" else wk_sb
            cs = slice(c * 512, (c + 1) * 512)
            st = {}

            def mk(kt):
                def _mm():
                    if "ps" not in st:
                        st["ps"] = psum.tile(
                            [128, 512], F32, name="ps_qk", tag="pj", bufs=1
                        )
                    nc.tensor.matmul(
                        st["ps"],
                        lhsT=wsb[:, kt, p * 128 : (p + 1) * 128],
                        rhs=xt_sb[:, kt, cs],
                        start=(kt == 0),
                        stop=(kt == KT - 1),
                    )
                return _mm

            def _copy():
                if kind == "q":
                    nc.vector.tensor_copy(out=qTp[p][:, cs], in_=st["ps"])
                    nc.vector.tensor_copy(
                        out=qTo[p][:, cs], in_=st["ps"][DH:128, :]
                    )
                else:
                    nc.vector.tensor_copy(out=kTp[p][:, cs], in_=st["ps"])
                qk_done.add((kind, p, c))

            return [(213, mk(kt)) for kt in range(KT)] + [(40, _copy)]

        def v_micros(it):
            st = {}

            def mk(kt):
                def _mm():
                    if "ps" not in st:
                        st["ps"] = psum.tile(
                            [128, 256], F32, name="vps", tag="pj", bufs=1
                        )
                    nc.tensor.matmul(
                        st["ps"],
                        lhsT=xt_sb[:, kt, it * 128 : (it + 1) * 128],
                        rhs=wv_sb[:, kt, :],
                        start=(kt == 0),
                        stop=(kt == KT - 1),
                    )
                return _mm

            def _evac():
                nc.vector.tensor_copy(
                    out=v_aug[:, it, :, 0:DH],
                    in_=st["ps"].rearrange("p (h d) -> p h d", h=HPC),
                )
                v_ready[it] = True

            return [(107, mk(kt)) for kt in range(KT)] + [(40, _evac)]

        def aoa_final_micros(h, half):
            """AoA linears (I+G packed in wcq/wca columns) + tanh-gate
            finals + output DMA for (head h, token half)."""
            p, odd = h // 2, h % 2
            st = {}
            hs = slice(half * 1024, (half + 1) * 1024)

            def mk_mm(cc, which):
                c = 2 * half + cc
                cs = slice(c * 512, (c + 1) * 512)

                def _mm():
                    if "ig" not in st:
                        st["ig"] = psum.tile(
                            [128, 1024], F32, name="ig", tag="pj", bufs=1
                        )
                    igs = st["ig"][:, cc * 512 : (cc + 1) * 512]
                    if which == 0:
                        assert ("q", p, c) in qk_done, f"aoa h{h} q c{c}"
                        qrhs = qTo[p][:, cs] if odd else qTp[p][0:DH, cs]
                        nc.tensor.matmul(
                            igs, lhsT=wcq_sb, rhs=qrhs, start=True, stop=False
                        )
                    else:
                        assert (h, c) in aoT_done, f"aoa h{h} aoT c{c}"
                        nc.tensor.matmul(
                            igs, lhsT=wca_sb, rhs=aoT[h][:, cs],
                            start=False, stop=True,
                        )
                return _mm

            def _tanh():
                st["y"] = fin.tile([DH, 1024], BF16, name="y", tag="y")
                nc.scalar.activation(
                    out=st["y"],
                    in_=st["ig"][DH:128, :],
                    func=AF.Tanh,
                    scale=0.5,
                    bias=bias_sb[DH : 2 * DH, :],
                )

            def _add1():
                st["y1"] = fin.tile([DH, 1024], BF16, name="y1", tag="y1")
                nc.vector.tensor_scalar_add(out=st["y1"], in0=st["y"], scalar1=1.0)

            def _stt():
                ot = fin.tile([DH, 1024], BF16, name="ot", tag="ot")
                nc.vector.scalar_tensor_tensor(
                    out=ot,
                    in0=st["ig"][0:DH, :],
                    scalar=bias_sb[0:DH, :],
                    in1=st["y1"],
                    op0=ALU.add,
                    op1=ALU.mult,
                )
                nc.sync.dma_start(out=outT_d[h * DH : (h + 1) * DH, hs], in_=ot)

            return [(213, mk_mm(cc, w)) for cc in range(2) for w in range(2)] + [
                (60, _tanh),
                (40, _add1),
                (40, _stt),
            ]

        def norm_chain(h, c):
            """aoT[h][:, chunk c] = ao_unnorm / L, off the critical path."""
            cs = slice(c * 512, (c + 1) * 512)
            rl = nrm.tile([1, 512], F32, name="rl", tag="rl")
            nc.vector.reciprocal(out=rl, in_=aoU[h][DH : DH + 1, cs])
            rlb = nrm.tile([DH, 512], F32, name="rlb", tag="rlb")
            nc.gpsimd.partition_broadcast(rlb, rl)
            nc.vector.tensor_mul(out=aoT[h][:, cs], in0=aoU[h][0:DH, cs], in1=rlb)
            aoT_done.add((h, c))

        # ---------------- pv chaser ----------------
        class Chaser:
            """Emits the PV accumulation for one (pair, chunk) lazily: PV(jt)
            requires exp(jt) emitted AND v tile jt produced. Accumulation
            over j is commutative, so a chunk's PV train may spill into
            later chunks' steps."""

            def __init__(self, p, c):
                self.p, self.c = p, c
                self.next = 0
                self.n_exp = 0
                self.es_tiles = [None] * JT
                self.pv = None

            def emit_one(self):
                jt = self.next
                if self.pv is None:
                    # lazy: ring slots recycle the previous chunk's tiles,
                    # whose evacs are emitted at that chaser's finish()
                    self.pv = [
                        psum.tile(
                            [DH + 1, 512], F32, name=f"pv{hh}", tag="pv", bufs=2
                        )
                        for hh in range(2)
                    ]
                for hh in range(2):
                    nc.tensor.matmul(
                        self.pv[hh],
                        lhsT=v_aug[:, jt, 2 * self.p + hh, :],
                        rhs=self.es_tiles[jt][:, hh * 512 : (hh + 1) * 512],
                        start=(jt == 0),
                        stop=(jt == JT - 1),
                    )
                self.es_tiles[jt] = None
                self.next += 1
                return 427

            def runnable(self):
                return (
                    self.next < self.n_exp
                    and self.next < JT
                    and v_ready[self.next]
                )

            def done(self):
                return self.next >= JT

            def finish(self):
                """Evacuate PSUM (both heads), norm chains, then queue the
                AoA+finals units whose aoT inputs just completed."""
                cs = slice(self.c * 512, (self.c + 1) * 512)
                for hh in range(2):
                    nc.vector.tensor_copy(
                        out=aoU[2 * self.p + hh][:, cs], in_=self.pv[hh]
                    )
                for hh in range(2):
                    norm_chain(2 * self.p + hh, self.c)
                if self.c % 2 == 1:
                    half = self.c // 2
                    for hh in range(2):
                        h = 2 * self.p + hh
                        add_unit(f"af{h}{half}", aoa_final_micros(h, half))

        chasers = deque()
        fillers = deque()
        emitted = set()

        def add_unit(name, micros):
            fillers.extend(micros)
            fillers.append((0, lambda: emitted.add(name)))

        def require(*names):
            """Emission order IS logical order for SBUF writes: force-pop
            fillers until the named units' writes are emitted."""
            for name in names:
                while name not in emitted:
                    if not fillers:
                        raise AssertionError(f"require({name}): fillers empty")
                    fillers.popleft()[1]()

        def es_outstanding():
            return sum(ch.n_exp - ch.next for ch in chasers)

        def force_chase_one():
            ch = chasers[0]
            if ch.done():
                ch.finish()
                chasers.popleft()
                return
            while not v_ready[ch.next]:
                if not fillers:
                    raise AssertionError("es forcing: fillers empty")
                fillers.popleft()[1]()
            ch.emit_one()

        def service(budget):
            while budget > 0 and chasers:
                ch = chasers[0]
                if ch.done():
                    ch.finish()
                    chasers.popleft()
                    continue
                if not ch.runnable():
                    break
                budget -= ch.emit_one()
            while fillers and budget >= fillers[0][0]:
                cost, fn = fillers.popleft()
                fn()
                budget -= cost
            return budget

        # ---------------- prologue: pair-0 chunk-0 q+k inline ----------
        for _, fn in qk_micros("q", 0, 0) + qk_micros("k", 0, 0):
            fn()
        emitted.update({"q00", "k00"})

        # ---------------- main exp stream ----------------
        for p in range(2):
            for c in range(NCH):
                if p == 0 and c == 0:
                    add_unit("k01", qk_micros("k", 0, 1))
                    for it in range(4):
                        add_unit(f"v{it}", v_micros(it))
                    add_unit("k02", qk_micros("k", 0, 2))
                    for it in range(4, 8):
                        add_unit(f"v{it}", v_micros(it))
                    add_unit("k03", qk_micros("k", 0, 3))
                    add_unit("q01", qk_micros("q", 0, 1))
                    for it in range(8, JT):
                        add_unit(f"v{it}", v_micros(it))
                elif p == 0 and c == 1:
                    add_unit("q02", qk_micros("q", 0, 2))
                elif p == 0 and c == 2:
                    add_unit("q03", qk_micros("q", 0, 3))
                elif p == 0 and c == 3:
                    add_unit("q10", qk_micros("q", 1, 0))
                    add_unit("k10", qk_micros("k", 1, 0))
                elif p == 1 and c == 0:
                    add_unit("k11", qk_micros("k", 1, 1))
                    add_unit("k12", qk_micros("k", 1, 2))
                    add_unit("k13", qk_micros("k", 1, 3))
                    add_unit("q11", qk_micros("q", 1, 1))
                elif p == 1 and c == 1:
                    add_unit("q12", qk_micros("q", 1, 2))
                elif p == 1 and c == 2:
                    add_unit("q13", qk_micros("q", 1, 3))

                require(f"q{p}{c}")
                ch = Chaser(p, c)
                chasers.append(ch)
                cs = slice(c * 512, (c + 1) * 512)
                for jt in range(JT):
                    require(f"k{p}{jt // 4}")
                    assert ("q", p, c) in qk_done and ("k", p, jt // 4) in qk_done
                    jts = slice(jt * 128, (jt + 1) * 128)
                    s = psum.tile([128, 1024], F32, name="s", tag="s", bufs=2)
                    nc.tensor.matmul(
                        s[:, 0:512],
                        lhsT=kTp[p][0:DH, jts],
                        rhs=qTp[p][0:DH, cs],
                        start=True,
                        stop=True,
                        tile_position=(0, 0),
                    )
                    nc.tensor.matmul(
                        s[:, 512:1024],
                        lhsT=kTp[p][DH:128, jts],
                        rhs=qTp[p][DH:128, cs],
                        start=True,
                        stop=True,
                        tile_position=(64, 0),
                    )
                    while chasers and es_outstanding() >= 11:
                        force_chase_one()
                    es = esp.tile([128, 1024], BF16, name="es", tag="es")
                    nc.scalar.activation(out=es, in_=s, func=AF.Exp, scale=SCALE)
                    ch.es_tiles[jt] = es
                    ch.n_exp = jt + 1
                    service(STEP_BUDGET - 427)

        # ---------------- drain + tail ----------------
        while chasers or fillers:
            progress = False
            while chasers:
                ch = chasers[0]
                if ch.done():
                    ch.finish()
                    chasers.popleft()
                    progress = True
                    continue
                if ch.runnable():
                    ch.emit_one()
                    progress = True
                    continue
                break
            if fillers:
                fillers.popleft()[1]()
                progress = True
            if not progress:
                raise AssertionError("drain deadlock")
    nc.compile()
    return nc


_NC_CACHE = None


def _get_nc():
    global _NC_CACHE
    if _NC_CACHE is None:
        _NC_CACHE = build_nc()
    return _NC_CACHE


def make_in_maps(x, Wq, Wkv, Wq_out, Wattn_out, out_bias, Wq_gate, Wattn_gate,
                 gate_bias):
    import ml_dtypes

    bf16 = ml_dtypes.bfloat16
    # sigmoid(x) = 0.5*(1+tanh(x/2)): fold the 0.5 into the I-half columns
    # and bias; G bias pre-halved for the ACT scale=0.5 path.
    wcq = np.ascontiguousarray(
        np.concatenate([0.5 * Wq_out.T, Wq_gate.T], axis=1), dtype=bf16
    )
    wca = np.ascontiguousarray(
        np.concatenate([0.5 * Wattn_out.T, Wattn_gate.T], axis=1), dtype=bf16
    )
    biases = 0.5 * np.concatenate(
        [out_bias.reshape(-1), gate_bias.reshape(-1)]
    ).astype(np.float32).reshape(2 * DH, 1)
    biases = np.ascontiguousarray(biases)
    Wk = Wkv[:, : H * DH]
    Wv = Wkv[:, H * DH :]
    xT = [
        np.ascontiguousarray(x[b].T.reshape(KT, 128, N)).astype(bf16)
        for b in range(B)
    ]
    in_maps = []
    for c in range(8):
        b, hg = c // 4, c % 4
        cols = slice(hg * INC, (hg + 1) * INC)
        in_maps.append(
            {
                "xT": xT[b],
                "wqkv": np.ascontiguousarray(
                    np.concatenate(
                        [Wq[:, cols], Wk[:, cols], Wv[:, cols]], axis=1
                    )
                ).astype(bf16),
                "wcq": wcq,
                "wca": wca,
                "biases": biases,
            }
        )
    return in_maps


def assemble_output(results):
    out = np.empty((B, N, H * DH), dtype=np.float32)
    for c in range(8):
        b, hg = c // 4, c % 4
        out[b, :, hg * INC : (hg + 1) * INC] = results[c]["outT"].T.astype(np.float32)
    return out


def kernel(**inputs):
    from concourse.bass_utils import run_bass_kernel_spmd

    inputs = {k: np.asarray(v, dtype=np.float32) for k, v in inputs.items()}
    nc = _get_nc()
    in_maps = make_in_maps(**inputs)
    res = run_bass_kernel_spmd(nc, in_maps, core_ids=list(range(8)))
    return assemble_output(res.results)


# revision 25
# speedup vs baseline: 1.6114x; 1.5345x over previous
"""AttentionOnAttention Trainium2 kernel v4 (8 NeuronCores, SPMD).

Sharding: core c handles batch b = c//4 and heads [4*(c%4), 4*(c%4)+4);
each core computes the disjoint output slice out[b, :, 256*(c%4):...] so no
collectives are needed.

Structure (kept from the 215.5us baseline, which measurement shows is at a
hardware equilibrium):
  - prefix: all q/k/v projections back-to-back (PE ramps to high p-state,
    ~0.48 ns/row effective; ACT idle here is unavoidable — in-stream
    filler micros measured ~2x more expensive due to the in-order PE
    queue + HAM activity throttle, so do NOT move projections into the
    stream);
  - exp stream: per j-step the row-packed S pair (tile_position (0,0)/
    (64,0) CO-EXECUTES, ~389ns) + pipelined PV pair (~591ns) exactly fill
    the ~1003ns ACT pace of one [128,1024] exp. No other PE work fits.
  - ACT table set loaded once (exp_and_others has Exp AND Tanh).

v4 deltas vs baseline:
  - finals use sigmoid(x) = 0.5*(1+tanh(x/2)) with 0.5 folded into the
    I-half AoA weights/bias host-side -> no sigmoid table switch (~2.7us)
    and finals can interleave with the stream;
  - AoA+finals for head-halves whose ao is complete are emitted as sparse
    fillers (1 micro/step) inside the LATER stream chunks instead of one
    serialized tail: pair-0 half-0 from p1c0, pair-0 half-1 from p1c1,
    pair-1 half-0 from p1c3; only pair-1 half-1 remains in the tail.
"""

import numpy as np
from collections import deque
from contextlib import ExitStack

import concourse.bass as bass
import concourse.bacc as bacc
import concourse.tile as tile
from concourse import mybir

B, N, DIM, H, DH = 2, 2048, 1024, 16, 64
HPC = H // 4          # 4 heads per core
INC = HPC * DH        # 256 per-core inner width
KT = DIM // 128       # 8 contraction tiles
NCH = N // 512        # 4 free-dim chunks of 512
JT = N // 128         # 16 j tiles
SCALE = float(DH) ** -0.5
F32 = mybir.dt.float32
BF16 = mybir.dt.bfloat16
AF = mybir.ActivationFunctionType
ALU = mybir.AluOpType


def build_nc():
    nc = bacc.Bacc(
        "TRN2",
        target_bir_lowering=False,
        debug=False,
        enable_asserts=False,
        num_devices=8,
    )
    xT_d = nc.dram_tensor("xT", (KT, 128, N), BF16, kind="ExternalInput").ap()
    wqkv_d = nc.dram_tensor("wqkv", (DIM, 3 * INC), BF16, kind="ExternalInput").ap()
    wcq_d = nc.dram_tensor("wcq", (DH, 2 * DH), BF16, kind="ExternalInput").ap()
    wca_d = nc.dram_tensor("wca", (DH, 2 * DH), BF16, kind="ExternalInput").ap()
    bias_d = nc.dram_tensor("biases", (2 * DH, 1), F32, kind="ExternalInput").ap()
    outT_d = nc.dram_tensor("outT", (INC, N), BF16, kind="ExternalOutput").ap()

    with tile.TileContext(nc) as tc, ExitStack() as ctx:
        consts = ctx.enter_context(tc.tile_pool(name="consts", bufs=1))
        psum = ctx.enter_context(tc.tile_pool(name="psum", bufs=3, space="PSUM"))

        # persistent tensors
        qTp = [consts.tile([128, N], BF16, name=f"qTp{p}") for p in range(2)]
        qTo = [consts.tile([DH, N], BF16, name=f"qTo{p}") for p in range(2)]
        kTp = [consts.tile([128, N], BF16, name=f"kTp{p}") for p in range(2)]
        v_aug = consts.tile([128, JT, HPC, DH + 1], BF16, name="v_aug")
        nc.vector.memset(v_aug[:, :, :, DH : DH + 1], 1.0)
        aoT = [consts.tile([DH, N], BF16, name=f"aoT{h}") for h in range(HPC)]
        aoU = [consts.tile([DH + 1, N], F32, name=f"aoU{h}") for h in range(HPC)]

        esp = ctx.enter_context(tc.tile_pool(name="es_p", bufs=8))
        nrm = ctx.enter_context(tc.tile_pool(name="norm_p", bufs=4))
        fin = ctx.enter_context(tc.tile_pool(name="fin_p", bufs=2))
        xw = tc.alloc_tile_pool(name="xw", bufs=1)
        # DMA order: first projection needs wq + wk + xt; wv afterwards.
        wqkv_sb = xw.tile([128, KT, 3 * INC], BF16, name="wqkv_sb")
        wq_sb = wqkv_sb[:, :, 0:INC]
        wk_sb = wqkv_sb[:, :, INC : 2 * INC]
        wv_sb = wqkv_sb[:, :, 2 * INC : 3 * INC]
        xt_sb = xw.tile([128, KT, N], BF16, name="xt_sb")
        for k in range(KT):
            ks = slice(k * 128, (k + 1) * 128)
            nc.sync.dma_start(out=wqkv_sb[:, k, :], in_=wqkv_d[ks, :])
            nc.sync.dma_start(out=xt_sb[:, k, 0:1024], in_=xT_d[k, :, 0:1024])
        for k in range(KT):
            nc.sync.dma_start(out=xt_sb[:, k, 1024:N], in_=xT_d[k, :, 1024:N])

        # AoA weights + biases: tiny DMAs kept off the critical early
        # dispatch slots (only needed from the attention tail onwards)
        wcq_sb = consts.tile([DH, 2 * DH], BF16, name="wcq_sb")
        nc.sync.dma_start(out=wcq_sb, in_=wcq_d)
        wca_sb = consts.tile([DH, 2 * DH], BF16, name="wca_sb")
        nc.sync.dma_start(out=wca_sb, in_=wca_d)
        bias_sb = consts.tile([2 * DH, 1], F32, name="bias_sb")
        nc.sync.dma_start(out=bias_sb, in_=bias_d)

        # Prefetch the exp/tanh ACT table set (one set serves the whole
        # kernel: exp stream + tanh gates) during the DMA prologue.
        warm_sb = consts.tile([128, 1], F32, name="warm_sb")
        nc.scalar.activation(out=warm_sb, in_=bias_sb, func=AF.Exp)
        nc.scalar.activation(out=warm_sb, in_=warm_sb, func=AF.Tanh)

        # ---------------- projection units ----------------
        def qk_unit(wsb, dst, p, c):
            cs = slice(c * 512, (c + 1) * 512)
            ps = psum.tile([128, 512], F32, name="ps_qk", tag="spair", bufs=3)
            for k in range(KT):
                nc.tensor.matmul(
                    ps,
                    lhsT=wsb[:, k, p * 128 : (p + 1) * 128],
                    rhs=xt_sb[:, k, cs],
                    start=(k == 0),
                    stop=(k == KT - 1),
                )
            nc.vector.tensor_copy(out=dst[p][:, cs], in_=ps)
            if dst is qTp:
                nc.vector.tensor_copy(out=qTo[p][:, cs], in_=ps[DH:128, :])

        def v_unit(it):
            psv = psum.tile([128, 512], F32, name="ps_v", tag="spair", bufs=3)
            for k in range(KT):
                nc.tensor.matmul(
                    psv[:, 0:INC],
                    lhsT=xt_sb[:, k, it * 128 : (it + 1) * 128],
                    rhs=wv_sb[:, k, :],
                    start=(k == 0),
                    stop=(k == KT - 1),
                )
            nc.vector.tensor_copy(
                out=v_aug[:, it, :, 0:DH],
                in_=psv[:, 0:INC].rearrange("p (h d) -> p h d", h=HPC),
            )

        # ---------------- AoA + finals micro units ----------------
        def aoa_final_micros(h, half):
            """AoA linears + tanh-gate finals + output DMA for (head h,
            token half), split into per-step filler micros."""
            p, odd = h // 2, h % 2
            st = {}
            hs = slice(half * 1024, (half + 1) * 1024)

            def mk_mm(cc, which):
                c = 2 * half + cc
                cs = slice(c * 512, (c + 1) * 512)

                def _mm():
                    if "ig" not in st:
                        st["ig"] = psum.tile(
                            [128, 1024], F32, name="ig", tag="spair", bufs=3
                        )
                    igs = st["ig"][:, cc * 512 : (cc + 1) * 512]
                    if which == 0:
                        qrhs = qTo[p][:, cs] if odd else qTp[p][0:DH, cs]
                        nc.tensor.matmul(
                            igs, lhsT=wcq_sb, rhs=qrhs, start=True, stop=False
                        )
                    else:
                        nc.tensor.matmul(
                            igs, lhsT=wca_sb, rhs=aoT[h][:, cs],
                            start=False, stop=True,
                        )
                return _mm

            def _tanh():
                st["y"] = fin.tile([DH, 1024], BF16, name="y", tag="y")
                nc.scalar.activation(
                    out=st["y"],
                    in_=st["ig"][DH:128, :],
                    func=AF.Tanh,
                    scale=0.5,
                    bias=bias_sb[DH : 2 * DH, :],
                )

            def _add1():
                st["y1"] = fin.tile([DH, 1024], BF16, name="y1", tag="y1")
                nc.vector.tensor_scalar_add(out=st["y1"], in0=st["y"], scalar1=1.0)

            def _stt():
                ot = fin.tile([DH, 1024], BF16, name="ot", tag="ot")
                nc.vector.scalar_tensor_tensor(
                    out=ot,
                    in0=st["ig"][0:DH, :],
                    scalar=bias_sb[0:DH, :],
                    in1=st["y1"],
                    op0=ALU.add,
                    op1=ALU.mult,
                )
                nc.sync.dma_start(out=outT_d[h * DH : (h + 1) * DH, hs], in_=ot)

            return [mk_mm(cc, w) for cc in range(2) for w in range(2)] + [
                _tanh,
                _add1,
                _stt,
            ]

        # ---------------- prefix: all projections ----------------
        for c in range(NCH):
            qk_unit(wq_sb, qTp, 0, c)
            qk_unit(wk_sb, kTp, 0, c)
            qk_unit(wq_sb, qTp, 1, c)
            qk_unit(wk_sb, kTp, 1, c)
            for it in range(4 * c, 4 * c + 4):
                v_unit(it)

        # ---------------- attention (+ sparse finals fillers) ----------
        def norm_chain(h, c):
            """aoT[h][:, chunk c] = ao_unnorm / L, off the critical path."""
            cs = slice(c * 512, (c + 1) * 512)
            rl = nrm.tile([1, 512], F32, name="rl", tag="rl")
            nc.vector.reciprocal(out=rl, in_=aoU[h][DH : DH + 1, cs])
            rlb = nrm.tile([DH, 512], F32, name="rlb", tag="rlb")
            nc.gpsimd.partition_broadcast(rlb, rl)
            nc.vector.tensor_mul(out=aoT[h][:, cs], in0=aoU[h][0:DH, cs], in1=rlb)

        fillers = deque()

        for p in range(2):
            for c in range(NCH):
                # queue finals for halves whose ao (all 4 chunks... only 2
                # needed) is fully normed before this chunk's steps begin:
                #  - (p1, c0): pair-0 half-0 (norms from p0c0/p0c1)
                #  - (p1, c1): pair-0 half-1 (norms from p0c2/p0c3)
                #  - (p1, c3): pair-1 half-0 (norms from p1c0/p1c1)
                if p == 1 and c == 0:
                    for h in (0, 1):
                        fillers.extend(aoa_final_micros(h, 0))
                elif p == 1 and c == 1:
                    for h in (0, 1):
                        fillers.extend(aoa_final_micros(h, 1))
                elif p == 1 and c == 3:
                    for h in (2, 3):
                        fillers.extend(aoa_final_micros(h, 0))

                cs = slice(c * 512, (c + 1) * 512)
                pv = [
                    psum.tile([DH + 1, 512], F32, name=f"pv{hh}", tag="pv", bufs=2)
                    for hh in range(2)
                ]
                es_tiles = [None] * JT

                def emit_pv(jt):
                    for hh in range(2):
                        nc.tensor.matmul(
                            pv[hh],
                            lhsT=v_aug[:, jt, 2 * p + hh, :],
                            rhs=es_tiles[jt][:, hh * 512 : (hh + 1) * 512],
                            start=(jt == 0),
                            stop=(jt == JT - 1),
                        )

                for jt in range(JT):
                    jts = slice(jt * 128, (jt + 1) * 128)
                    s = psum.tile([128, 1024], F32, name="s", tag="spair", bufs=3)
                    nc.tensor.matmul(
                        s[:, 0:512],
                        lhsT=kTp[p][0:DH, jts],
                        rhs=qTp[p][0:DH, cs],
                        start=True,
                        stop=True,
                        tile_position=(0, 0),
                    )
                    nc.tensor.matmul(
                        s[:, 512:1024],
                        lhsT=kTp[p][DH:128, jts],
                        rhs=qTp[p][DH:128, cs],
                        start=True,
                        stop=True,
                        tile_position=(64, 0),
                    )
                    es = esp.tile([128, 1024], BF16, name="es", tag="es")
                    nc.scalar.activation(out=es, in_=s, func=AF.Exp, scale=SCALE)
                    es_tiles[jt] = es
                    # keep PE one S-tile ahead of the PV consumer
                    if jt > 0:
                        emit_pv(jt - 1)
                    # at most ONE finals micro per step: measured slack in
                    # the steady state is tiny; a single small op per step
                    # rides the exp pace without queue-blocking the next S
                    if fillers:
                        fillers.popleft()()

                emit_pv(JT - 1)

                # evacuate PSUM quickly (both heads), THEN the slow norm
                # chains — otherwise the reciprocal blocks the second copy
                # on the in-order DVE and the next chunk's PV stalls on PSUM
                for hh in range(2):
                    nc.vector.tensor_copy(out=aoU[2 * p + hh][:, cs], in_=pv[hh])
                if not (p == 1 and c == NCH - 1):
                    for hh in range(2):
                        norm_chain(2 * p + hh, c)

            if p == 0:
                xw.release()

        # ---------------- tail: pair-1 half-1 ----------
        # The cc=0 AoA matmuls (reading chunk-2 aoT, already normed) are
        # emitted BEFORE the deferred last-chunk norms so TensorE isn't
        # blocked behind the GPSIMD broadcasts.
        while fillers:
            fillers.popleft()()
        af2 = aoa_final_micros(2, 1)
        af3 = aoa_final_micros(3, 1)
        af2[0](); af2[1](); af3[0](); af3[1]()      # cc=0 mms (dep-free)
        norm_chain(2, NCH - 1)
        af2[2](); af2[3]()                          # cc=1 mms head 2
        norm_chain(3, NCH - 1)
        for fn in af2[4:]:                          # tanh/add1/stt head 2
            fn()
        for fn in af3[2:]:                          # cc=1 mms + finals head 3
            fn()
    nc.compile()
    return nc


_NC_CACHE = None


def _get_nc():
    global _NC_CACHE
    if _NC_CACHE is None:
        _NC_CACHE = build_nc()
    return _NC_CACHE


def make_in_maps(x, Wq, Wkv, Wq_out, Wattn_out, out_bias, Wq_gate, Wattn_gate,
                 gate_bias):
    import ml_dtypes

    bf16 = ml_dtypes.bfloat16
    # sigmoid(x) = 0.5*(1+tanh(x/2)): fold the 0.5 into the I-half columns
    # and bias; G bias pre-halved for the ACT scale=0.5 path.
    wcq = np.ascontiguousarray(
        np.concatenate([0.5 * Wq_out.T, Wq_gate.T], axis=1), dtype=bf16
    )
    wca = np.ascontiguousarray(
        np.concatenate([0.5 * Wattn_out.T, Wattn_gate.T], axis=1), dtype=bf16
    )
    biases = 0.5 * np.concatenate(
        [out_bias.reshape(-1), gate_bias.reshape(-1)]
    ).astype(np.float32).reshape(2 * DH, 1)
    biases = np.ascontiguousarray(biases)
    Wk = Wkv[:, : H * DH]
    Wv = Wkv[:, H * DH :]
    xT = [
        np.ascontiguousarray(x[b].T.reshape(KT, 128, N)).astype(bf16)
        for b in range(B)
    ]
    in_maps = []
    for c in range(8):
        b, hg = c // 4, c % 4
        cols = slice(hg * INC, (hg + 1) * INC)
        in_maps.append(
            {
                "xT": xT[b],
                "wqkv": np.ascontiguousarray(
                    np.concatenate(
                        [Wq[:, cols], Wk[:, cols], Wv[:, cols]], axis=1
                    )
                ).astype(bf16),
                "wcq": wcq,
                "wca": wca,
                "biases": biases,
            }
        )
    return in_maps


def assemble_output(results):
    out = np.empty((B, N, H * DH), dtype=np.float32)
    for c in range(8):
        b, hg = c // 4, c % 4
        out[b, :, hg * INC : (hg + 1) * INC] = results[c]["outT"].T.astype(np.float32)
    return out


def kernel(**inputs):
    from concourse.bass_utils import run_bass_kernel_spmd

    inputs = {k: np.asarray(v, dtype=np.float32) for k, v in inputs.items()}
    nc = _get_nc()
    in_maps = make_in_maps(**inputs)
    res = run_bass_kernel_spmd(nc, in_maps, core_ids=list(range(8)))
    return assemble_output(res.results)


# revision 26
# speedup vs baseline: 1.7070x; 1.0593x over previous
"""Original staged baseline kernel (217.7us/215.5us) — for A/B control runs."""

import numpy as np
from contextlib import ExitStack

import concourse.bass as bass
import concourse.bacc as bacc
import concourse.tile as tile
from concourse import mybir

B, N, DIM, H, DH = 2, 2048, 1024, 16, 64
HPC = H // 4          # 4 heads per core
INC = HPC * DH        # 256 per-core inner width
KT = DIM // 128       # 8 contraction tiles
NCH = N // 512        # 4 free-dim chunks of 512
JT = N // 128         # 16 j tiles
SCALE = float(DH) ** -0.5
F32 = mybir.dt.float32
BF16 = mybir.dt.bfloat16
AF = mybir.ActivationFunctionType
ALU = mybir.AluOpType


def build_nc():
    nc = bacc.Bacc(
        "TRN2",
        target_bir_lowering=False,
        debug=False,
        enable_asserts=False,
        num_devices=8,
    )
    xT_d = nc.dram_tensor("xT", (KT, 128, N), BF16, kind="ExternalInput").ap()
    wqkv_d = nc.dram_tensor("wqkv", (DIM, 3 * INC), BF16, kind="ExternalInput").ap()
    wcq_d = nc.dram_tensor("wcq", (DH, 2 * DH), BF16, kind="ExternalInput").ap()
    wca_d = nc.dram_tensor("wca", (DH, 2 * DH), BF16, kind="ExternalInput").ap()
    bias_d = nc.dram_tensor("biases", (2 * DH, 1), F32, kind="ExternalInput").ap()
    outT_d = nc.dram_tensor("outT", (INC, N), BF16, kind="ExternalOutput").ap()

    with tile.TileContext(nc) as tc, ExitStack() as ctx:
        consts = ctx.enter_context(tc.tile_pool(name="consts", bufs=1))
        psum = ctx.enter_context(tc.tile_pool(name="psum", bufs=3, space="PSUM"))

        # persistent tensors
        qTp = [consts.tile([128, N], BF16, name=f"qTp{p}") for p in range(2)]
        qTo = [consts.tile([DH, N], BF16, name=f"qTo{p}") for p in range(2)]
        kTp = [consts.tile([128, N], BF16, name=f"kTp{p}") for p in range(2)]
        v_aug = consts.tile([128, JT, HPC, DH + 1], BF16, name="v_aug")
        nc.vector.memset(v_aug[:, :, :, DH : DH + 1], 1.0)
        aoT = [consts.tile([DH, N], BF16, name=f"aoT{h}") for h in range(HPC)]
        aoU = [consts.tile([DH + 1, N], F32, name=f"aoU{h}") for h in range(HPC)]

        esp = ctx.enter_context(tc.tile_pool(name="es_p", bufs=8))
        nrm = ctx.enter_context(tc.tile_pool(name="norm_p", bufs=4))
        xw = tc.alloc_tile_pool(name="xw", bufs=1)
        # DMA order: first projection needs wq + wk + xt; wv afterwards.
        wqkv_sb = xw.tile([128, KT, 3 * INC], BF16, name="wqkv_sb")
        wq_sb = wqkv_sb[:, :, 0:INC]
        wk_sb = wqkv_sb[:, :, INC : 2 * INC]
        wv_sb = wqkv_sb[:, :, 2 * INC : 3 * INC]
        xt_sb = xw.tile([128, KT, N], BF16, name="xt_sb")
        for k in range(KT):
            ks = slice(k * 128, (k + 1) * 128)
            nc.sync.dma_start(out=wqkv_sb[:, k, :], in_=wqkv_d[ks, :])
            nc.sync.dma_start(out=xt_sb[:, k, 0:1024], in_=xT_d[k, :, 0:1024])
        for k in range(KT):
            nc.sync.dma_start(out=xt_sb[:, k, 1024:N], in_=xT_d[k, :, 1024:N])

        # AoA weights + biases: tiny DMAs kept off the critical early
        # dispatch slots (only needed from the attention tail onwards)
        wcq_sb = consts.tile([DH, 2 * DH], BF16, name="wcq_sb")
        nc.sync.dma_start(out=wcq_sb, in_=wcq_d)
        wca_sb = consts.tile([DH, 2 * DH], BF16, name="wca_sb")
        nc.sync.dma_start(out=wca_sb, in_=wca_d)
        bias_sb = consts.tile([2 * DH, 1], F32, name="bias_sb")
        nc.sync.dma_start(out=bias_sb, in_=bias_d)

        # Prefetch the exp/tanh ACT table set during the DMA prologue so the
        # first attention exp doesn't stall PE long enough to re-throttle HAM.
        warm_sb = consts.tile([128, 1], F32, name="warm_sb")
        nc.scalar.activation(out=warm_sb, in_=bias_sb, func=AF.Exp)
        nc.scalar.activation(out=warm_sb, in_=warm_sb, func=AF.Tanh)

        # ---------------- projection units ----------------
        def qk_unit(wsb, dst, p, c):
            for m in qk_micros(wsb, dst, p, c):
                m()

        def qk_micros(wsb, dst, p, c):
            """One closure per matmul/copy so filler work can be spread one
            instruction per attention step (PE slack is ~0.4us/step)."""
            cs = slice(c * 512, (c + 1) * 512)
            state = {}

            def mk_mm(k):
                def _mm():
                    if "ps" not in state:
                        state["ps"] = psum.tile(
                            [128, 512], F32, name="ps_qk", tag="spair", bufs=3
                        )
                    nc.tensor.matmul(
                        state["ps"],
                        lhsT=wsb[:, k, p * 128 : (p + 1) * 128],
                        rhs=xt_sb[:, k, cs],
                        start=(k == 0),
                        stop=(k == KT - 1),
                    )
                return _mm

            def _copy():
                ps = state["ps"]
                nc.vector.tensor_copy(out=dst[p][:, cs], in_=ps)
                if dst is qTp:
                    nc.vector.tensor_copy(out=qTo[p][:, cs], in_=ps[DH:128, :])

            return [mk_mm(k) for k in range(KT)] + [_copy]

        def v_unit(it):
            psv = psum.tile([128, 512], F32, name="ps_v", tag="spair", bufs=3)
            for k in range(KT):
                nc.tensor.matmul(
                    psv[:, 0:INC],
                    lhsT=xt_sb[:, k, it * 128 : (it + 1) * 128],
                    rhs=wv_sb[:, k, :],
                    start=(k == 0),
                    stop=(k == KT - 1),
                )
            nc.vector.tensor_copy(
                out=v_aug[:, it, :, 0:DH],
                in_=psv[:, 0:INC].rearrange("p (h d) -> p h d", h=HPC),
            )

        def aoa_half(h, half):
            p = h // 2
            odd = h % 2
            ig = psum.tile([128, 1024], F32, name="igh", tag="spair", bufs=3)
            for cc in range(2):
                c = 2 * half + cc
                cs = slice(c * 512, (c + 1) * 512)
                igs = ig[:, cc * 512 : (cc + 1) * 512]
                qrhs = qTo[p][:, cs] if odd else qTp[p][0:DH, cs]
                nc.tensor.matmul(igs, lhsT=wcq_sb, rhs=qrhs, start=True, stop=False)
                nc.tensor.matmul(
                    igs, lhsT=wca_sb, rhs=aoT[h][:, cs], start=False, stop=True
                )
            return ig

        # ---------------- prefix: pair-0 q/k + v tiles 0..7 ----------------
        for c in range(NCH):
            qk_unit(wq_sb, qTp, 0, c)
            qk_unit(wk_sb, kTp, 0, c)
            qk_unit(wq_sb, qTp, 1, c)
            qk_unit(wk_sb, kTp, 1, c)
            for it in range(4 * c, 4 * c + 4):
                v_unit(it)

        # ---------------- attention (+ fillers) ----------------
        def norm_chain(h, c):
            """aoT[h][:, chunk c] = ao_unnorm / L, off the critical path."""
            cs = slice(c * 512, (c + 1) * 512)
            rl = nrm.tile([1, 512], F32, name="rl", tag="rl")
            nc.vector.reciprocal(out=rl, in_=aoU[h][DH : DH + 1, cs])
            rlb = nrm.tile([DH, 512], F32, name="rlb", tag="rlb")
            nc.gpsimd.partition_broadcast(rlb, rl)
            nc.vector.tensor_mul(out=aoT[h][:, cs], in0=aoU[h][0:DH, cs], in1=rlb)

        for p in range(2):

            for c in range(NCH):
                cs = slice(c * 512, (c + 1) * 512)
                pv = [
                    psum.tile([DH + 1, 512], F32, name=f"pv{hh}", tag="pv", bufs=2)
                    for hh in range(2)
                ]
                es_tiles = [None] * JT

                def emit_pv(jt):
                    for hh in range(2):
                        nc.tensor.matmul(
                            pv[hh],
                            lhsT=v_aug[:, jt, 2 * p + hh, :],
                            rhs=es_tiles[jt][:, hh * 512 : (hh + 1) * 512],
                            start=(jt == 0),
                            stop=(jt == JT - 1),
                        )

                for jt in range(JT):
                    jts = slice(jt * 128, (jt + 1) * 128)
                    s = psum.tile([128, 1024], F32, name="s", tag="spair", bufs=3)
                    nc.tensor.matmul(
                        s[:, 0:512],
                        lhsT=kTp[p][0:DH, jts],
                        rhs=qTp[p][0:DH, cs],
                        start=True,
                        stop=True,
                        tile_position=(0, 0),
                    )
                    nc.tensor.matmul(
                        s[:, 512:1024],
                        lhsT=kTp[p][DH:128, jts],
                        rhs=qTp[p][DH:128, cs],
                        start=True,
                        stop=True,
                        tile_position=(64, 0),
                    )
                    es = esp.tile([128, 1024], BF16, name="es", tag="es")
                    nc.scalar.activation(out=es, in_=s, func=AF.Exp, scale=SCALE)
                    es_tiles[jt] = es
                    # keep PE one S-tile ahead of the PV consumer
                    if jt > 0:
                        emit_pv(jt - 1)

                emit_pv(JT - 1)

                # evacuate PSUM quickly (both heads), THEN the slow norm
                # chains — otherwise the reciprocal blocks the second copy on
                # the in-order DVE and the next i-chunk's PV stalls on PSUM
                hh_order = (1, 0) if (p == 1 and c == NCH - 1) else (0, 1)
                for hh in hh_order:
                    nc.vector.tensor_copy(out=aoU[2 * p + hh][:, cs], in_=pv[hh])
                if not (p == 1 and c == NCH - 1):
                    for hh in range(2):
                        norm_chain(2 * p + hh, c)

            if p == 0:
                xw.release()

        # ---------------- AoA + finals ----------------
        fin = ctx.enter_context(tc.tile_pool(name="fin_p", bufs=8))

        def finals_half(h, half, ig=None):
            if ig is None:
                ig = aoa_half(h, half)
            hs = slice(half * 1024, (half + 1) * 1024)
            sgm = fin.tile([DH, 1024], F32, name="sgm", tag="sgm")
            nc.scalar.activation(
                out=sgm,
                in_=ig[DH:128, :],
                func=AF.Sigmoid,
                scale=1.0,
                bias=bias_sb[DH : 2 * DH, :],
            )
            ot = fin.tile([DH, 1024], BF16, name="ot", tag="ot")
            nc.vector.scalar_tensor_tensor(
                out=ot,
                in0=ig[0:DH, :],
                scalar=bias_sb[0:DH, :],
                in1=sgm,
                op0=ALU.add,
                op1=ALU.mult,
            )
            nc.sync.dma_start(out=outT_d[h * DH : (h + 1) * DH, hs], in_=ot)

        nc.scalar.activation(out=warm_sb[0:1, :], in_=warm_sb[0:1, :],
                             func=AF.Sigmoid)
        ig00 = aoa_half(0, 0)
        ig01 = aoa_half(0, 1)
        ig10 = aoa_half(1, 0)
        norm_chain(3, NCH - 1)
        finals_half(0, 0, ig00)
        finals_half(0, 1, ig01)
        norm_chain(2, NCH - 1)
        finals_half(1, 0, ig10)
        finals_half(1, 1)
        finals_half(3, 0)
        finals_half(3, 1)
        finals_half(2, 0)
        finals_half(2, 1)
    nc.compile()
    return nc


_NC_CACHE = None


def _get_nc():
    global _NC_CACHE
    if _NC_CACHE is None:
        _NC_CACHE = build_nc()
    return _NC_CACHE


def make_in_maps(x, Wq, Wkv, Wq_out, Wattn_out, out_bias, Wq_gate, Wattn_gate,
                 gate_bias):
    import ml_dtypes

    bf16 = ml_dtypes.bfloat16
    wcq = np.ascontiguousarray(np.concatenate([Wq_out.T, Wq_gate.T], axis=1),
                               dtype=bf16)
    wca = np.ascontiguousarray(
        np.concatenate([Wattn_out.T, Wattn_gate.T], axis=1), dtype=bf16
    )
    biases = np.concatenate(
        [out_bias.reshape(-1), gate_bias.reshape(-1)]
    ).astype(np.float32).reshape(2 * DH, 1)
    biases = np.ascontiguousarray(biases)
    Wk = Wkv[:, : H * DH]
    Wv = Wkv[:, H * DH :]
    xT = [
        np.ascontiguousarray(x[b].T.reshape(KT, 128, N)).astype(bf16)
        for b in range(B)
    ]
    in_maps = []
    for c in range(8):
        b, hg = c // 4, c % 4
        cols = slice(hg * INC, (hg + 1) * INC)
        in_maps.append(
            {
                "xT": xT[b],
                "wqkv": np.ascontiguousarray(
                    np.concatenate(
                        [Wq[:, cols], Wk[:, cols], Wv[:, cols]], axis=1
                    )
                ).astype(bf16),
                "wcq": wcq,
                "wca": wca,
                "biases": biases,
            }
        )
    return in_maps


def assemble_output(results):
    out = np.empty((B, N, H * DH), dtype=np.float32)
    for c in range(8):
        b, hg = c // 4, c % 4
        out[b, :, hg * INC : (hg + 1) * INC] = results[c]["outT"].T.astype(np.float32)
    return out


def kernel(**inputs):
    from concourse.bass_utils import run_bass_kernel_spmd

    inputs = {k: np.asarray(v, dtype=np.float32) for k, v in inputs.items()}
    nc = _get_nc()
    in_maps = make_in_maps(**inputs)
    res = run_bass_kernel_spmd(nc, in_maps, core_ids=list(range(8)))
    return assemble_output(res.results)
